# revision 7
# baseline (speedup 1.0000x reference)
"""Trainium2 Bass kernel for nn_AttentionWithTime — fp8 DoubleRow redesign.

Data-parallel over batch B=8 (one element per core). See per-stage comments.

Speed structure (cost-model driven):
  - every large matmul runs fp8(e4m3) DoubleRow: two 128-row k-subtiles per
    instruction at 0.5 cycles/output-row
  - scores contract DH=64 as two 32-row subtiles ([32,2,N] packs built by
    SBUF->SBUF DMA repartition of the projection output)
  - softmax transpose+normalize+differential-combine fused into ONE fp8 DR
    matmul per 128x128 chunk: psum = E1^T diag(1024/d1) + E2^T diag(-1024
    lam/d2) -- a diagonal rhs applies per-query scaling during the transpose,
    the DR subtile pair does the subtract
  - merge folded into attention: VW_h = v_h @ Wm_h precomputed, then x2
    accumulates U_h @ VW_h over ALL heads+keys in one psum per token tile
  - v-bias via attn row-sum identity: rows sum to (1-lam), so the effect is
    the fixed row (1-lam)*(b_v@Wm) folded into the broadcast residual rows
  - LN feature transposes via DMA xbar (bf16); gain/bias fused into the fp8
    pack cast; LN sqrt batched (one ACT Sqrt per LN stage -> 4 table loads)
  - weight fp8 casts (x32/x16 scales, descaled in later psum evacuations)
    spread across ACT (startup-idle) / DVE / Pool
  - emission order software-pipelines: x+LN1 first, qk weights -> proj ->
    repack, v_fm/VW interleaved into early attention iterations, ffn weight
    staging during attention
"""
import numpy as np

import concourse.bass as bass
import concourse.mybir as mybir
import concourse.tile as tile
from concourse import bacc
from concourse.masks import make_identity

B, N, D, H, DH, DE, DT = 8, 1024, 512, 8, 64, 2048, 256
DQKV = 6144
NT = N // 128
FT = D // 128
EPS = 1e-5
SCALE = DH ** -0.5

f32 = mybir.dt.float32
bf16 = mybir.dt.bfloat16
fp8 = mybir.dt.float8e4
AF = mybir.ActivationFunctionType
ALU = mybir.AluOpType
DR = mybir.MatmulPerfMode.DoubleRow


def build_program(lam: float, ln_trivial: bool = False):
    nc = bacc.Bacc("TRN2", target_bir_lowering=False, debug=False, num_devices=8)

    x_d = nc.dram_tensor("x", [N, D], f32, kind="ExternalInput")
    t_d = nc.dram_tensor("t", [DT], f32, kind="ExternalInput")
    Wqkv_d = nc.dram_tensor("Wqkv", [D, DQKV], f32, kind="ExternalInput")
    bqkv_d = nc.dram_tensor("bqkv", [DQKV], f32, kind="ExternalInput")
    Wm_d = nc.dram_tensor("Wm", [4096, D], f32, kind="ExternalInput")
    bm_d = nc.dram_tensor("bm", [D], f32, kind="ExternalInput")
    Wt1_d = nc.dram_tensor("Wt1", [DT, DT], f32, kind="ExternalInput")
    bt1_d = nc.dram_tensor("bt1", [DT], f32, kind="ExternalInput")
    Wt2_d = nc.dram_tensor("Wt2", [DT, D], f32, kind="ExternalInput")
    bt2_d = nc.dram_tensor("bt2", [D], f32, kind="ExternalInput")
    Wf1_d = nc.dram_tensor("Wf1", [D, DE], f32, kind="ExternalInput")
    bf1_d = nc.dram_tensor("bf1", [DE], f32, kind="ExternalInput")
    Wf2_d = nc.dram_tensor("Wf2", [DE, D], f32, kind="ExternalInput")
    bf2_d = nc.dram_tensor("bf2", [D], f32, kind="ExternalInput")
    ln1g_d = nc.dram_tensor("ln1_g", [D], f32, kind="ExternalInput")
    ln1b_d = nc.dram_tensor("ln1_b", [D], f32, kind="ExternalInput")
    lnfg_d = nc.dram_tensor("lnf_g", [D], f32, kind="ExternalInput")
    lnfb_d = nc.dram_tensor("lnf_b", [D], f32, kind="ExternalInput")
    y_d = nc.dram_tensor("y", [N, D], f32, kind="ExternalOutput")

    with tile.TileContext(nc) as tc:
        _build(tc, lam, ln_trivial, locals())
    nc.compile()
    return nc


def _build(tc, lam, ln_trivial, d):
    nc = tc.nc
    x_d, t_d, y_d = d["x_d"], d["t_d"], d["y_d"]
    Wqkv_d, bqkv_d, Wm_d, bm_d = d["Wqkv_d"], d["bqkv_d"], d["Wm_d"], d["bm_d"]
    Wt1_d, bt1_d, Wt2_d, bt2_d = d["Wt1_d"], d["bt1_d"], d["Wt2_d"], d["bt2_d"]
    Wf1_d, bf1_d, Wf2_d, bf2_d = d["Wf1_d"], d["bf1_d"], d["Wf2_d"], d["bf2_d"]
    ln1g_d, ln1b_d, lnfg_d, lnfb_d = d["ln1g_d"], d["ln1b_d"], d["lnfg_d"], d["lnfb_d"]

    dma = nc.sync.dma_start

    from contextlib import ExitStack
    with ExitStack() as es:
        cst = es.enter_context(tc.tile_pool(name="cst", bufs=1))
        small = es.enter_context(tc.tile_pool(name="small", bufs=8))
        xp = es.enter_context(tc.tile_pool(name="xp", bufs=NT))
        x2p = es.enter_context(tc.tile_pool(name="x2p", bufs=NT))
        ps_m = es.enter_context(tc.tile_pool(name="ps_m", bufs=2, space="PSUM"))

        # ---------- tiny constants ----------
        ident = cst.tile([128, 128], f32)
        make_identity(nc, ident[:])
        ident64 = cst.tile([128, 128], fp8)
        nc.gpsimd.tensor_scalar(ident64[:], ident[:], 64.0, None, ALU.mult)
        identneg = cst.tile([128, 128], fp8)
        nc.gpsimd.tensor_scalar(identneg[:], ident[:], -64.0 * lam, None, ALU.mult)
        ones1 = cst.tile([1, 128], f32)
        nc.gpsimd.memset(ones1[:], 1.0)
        eps_c = cst.tile([128, 1], f32)
        nc.gpsimd.memset(eps_c[:], EPS)

        ln1g_c = cst.tile([128, FT], f32)
        ln1b_c = cst.tile([128, FT], f32)
        lnfg_c = cst.tile([128, FT], f32)
        lnfb_c = cst.tile([128, FT], f32)
        bf1_c = cst.tile([128, DE // 128], f32)
        bt1_c = cst.tile([128, DT // 128], f32)
        bqp = cst.tile([128, 16], f32)
        bv8 = cst.tile([128, 32, 1], fp8)
        tT = cst.tile([128, 2], f32)
        bm_r = cst.tile([1, D], f32)
        bt2_r = cst.tile([1, D], f32)
        bf2_r = cst.tile([1, D], f32)
        row1 = cst.tile([1, D], f32)
        row21 = cst.tile([1, D], f32)
        TP1 = cst.tile([128, D], f32)
        row21b = cst.tile([1, D], bf16)
        ones1b = cst.tile([1, 128], bf16)
        mv1 = cst.tile([128, 16], f32)   # LN1 (mean,var) per nt
        sr1 = cst.tile([128, 16], f32)   # LN1 (rstd, -mean*rstd) per nt... [0:8]=rstd,[8:16]=nm
        mvf = cst.tile([128, 16], f32)
        srf = cst.tile([128, 16], f32)

        xts, x2ts = [], []

        # ---------- x DMAs + LN1 stats (first in the DMA queue) ----------
        with tc.tile_pool(name="xstage", bufs=3) as xst:
            for nt in range(NT):
                xf = xst.tile([128, D], f32, tag="xf", name="xf")
                dma(xf[:], x_d[nt * 128:(nt + 1) * 128, :])
                xt = xp.tile([128, D], bf16, name=f"xt_{nt}", tag="xt")
                nc.vector.tensor_copy(xt[:], xf[:])
                xts.append(xt)
                st6 = small.tile([128, 6], f32, tag="st6")
                nc.vector.bn_stats(out=st6[:], in_=xf[:])
                nc.vector.bn_aggr(out=mv1[:, 2 * nt:2 * nt + 2], in_=st6[:])

        # ---------- bias rows / columns ----------
        dma(bm_r[:], bm_d[:].rearrange("(o k) -> o k", o=1))
        dma(bt2_r[:], bt2_d[:].rearrange("(o k) -> o k", o=1))
        dma(bf2_r[:], bf2_d[:].rearrange("(o k) -> o k", o=1))
        with tc.tile_pool(name="rows", bufs=1) as rows, \
             tc.tile_pool(name="ps_tr", bufs=2, space="PSUM") as ps_tr:

            def row_to_cols(row, dest, pairs):
                for off, plen, pb, col in pairs:
                    trp = ps_tr.tile([128, 128], f32, tag="tr", name="rc_tr")
                    nc.tensor.transpose(trp[pb:pb + plen, 0:1], row[0:1, off:off + plen],
                                        ones1[0:1, 0:1])
                    nc.vector.tensor_copy(dest[pb:pb + plen, col:col + 1],
                                          trp[pb:pb + plen, 0:1])

            bq_row = rows.tile([1, DQKV], f32)
            dma(bq_row[:], bqkv_d[:].rearrange("(o k) -> o k", o=1))
            lng_row = rows.tile([1, 4 * D], f32)
            for i, v_d in enumerate((ln1g_d, ln1b_d, lnfg_d, lnfb_d)):
                dma(lng_row[0:1, i * D:(i + 1) * D], v_d[:].rearrange("(o k) -> o k", o=1))
            bf1_row = rows.tile([1, DE], f32)
            dma(bf1_row[:], bf1_d[:].rearrange("(o k) -> o k", o=1))
            bt1_row = rows.tile([1, DT], f32)
            dma(bt1_row[:], bt1_d[:].rearrange("(o k) -> o k", o=1))
            t_row = rows.tile([1, DT], f32)
            dma(t_row[:], t_d[:].rearrange("(o k) -> o k", o=1))
            for i, dest in enumerate((ln1g_c, ln1b_c, lnfg_c, lnfb_c)):
                row_to_cols(lng_row, dest, [(i * D + j * 128, 128, 0, j) for j in range(FT)])
            row_to_cols(bf1_row, bf1_c, [(j * 128, 128, 0, j) for j in range(DE // 128)])
            row_to_cols(bt1_row, bt1_c, [(j * 128, 128, 0, j) for j in range(DT // 128)])
            for p in range(16):
                h, base = p % 8, (0 if p < 8 else 1024)
                sr = rows.tile([1, 128], f32, tag="pair_row", bufs=3, name="pair_row")
                nc.gpsimd.tensor_copy(sr[0:1, 0:64], bq_row[0:1, base + h * 64:base + (h + 1) * 64])
                nc.gpsimd.tensor_copy(sr[0:1, 64:128], bq_row[0:1, base + 512 + h * 64:base + 512 + (h + 1) * 64])
                row_to_cols(sr, bqp, [(0, 128, 0, p)])
            bvv = rows.tile([128, 32], f32, tag="bvv", name="bvv")
            row_to_cols(bq_row, bvv, [(2048 + j * 128, 128, 0, j) for j in range(32)])
            nc.vector.tensor_scalar(bv8[:].rearrange("p c o -> p (c o)"), bvv[:],
                                    32.0, None, ALU.mult)
            row_to_cols(t_row, tT, [(j * 128, 128, 0, j) for j in range(2)])
            nc.vector.tensor_scalar(bqp[:], bqp[:], 32.0, None, ALU.mult)

            # ---------- time MLP (Silu table first) ----------
            wt1 = [rows.tile([128, DT], f32, name=f"wt1_{i}", tag="wt1") for i in range(2)]
            wt2 = [rows.tile([128, D], f32, name=f"wt2_{i}", tag="wt2") for i in range(2)]
            for ft in range(2):
                dma(wt1[ft][:], Wt1_d[ft * 128:(ft + 1) * 128, :])
                dma(wt2[ft][:], Wt2_d[ft * 128:(ft + 1) * 128, :])
            s_cols = []
            for dc in range(2):
                l1_ps = ps_tr.tile([128, 512], f32, tag="tmlp")
                for ft in range(2):
                    nc.tensor.matmul(l1_ps[:, 0:1], wt1[ft][:, dc * 128:(dc + 1) * 128],
                                     tT[:, ft:ft + 1], start=(ft == 0), stop=(ft == 1))
                s_c = small.tile([128, 1], f32, tag="s_col")
                nc.scalar.activation(s_c[:], l1_ps[:, 0:1], AF.Silu, bias=bt1_c[:, dc:dc + 1])
                s_cols.append(s_c)
            tp_ps = ps_tr.tile([1, 512], f32, tag="tmlp")
            for dc in range(2):
                nc.tensor.matmul(tp_ps[:], s_cols[dc][:], wt2[dc][:],
                                 start=(dc == 0), stop=(dc == 1))
            nc.vector.tensor_add(row1[:], tp_ps[:], bt2_r[:])
            nc.vector.tensor_sub(row21[:], bf2_r[:], row1[:])
            nc.vector.tensor_add(row1[:], row1[:], bm_r[:])

        # ---------- LN1: one batched Sqrt, then normalize+transpose+pack ----
        # sr1[:,0:8]=rstd, sr1[:,8:16]=-mean*rstd
        mvv = mv1[:].rearrange("p (n c) -> p n c", c=2)
        nc.scalar.activation(sr1[:, 0:8], mvv[:, :, 1], AF.Sqrt, bias=eps_c[:])
        nc.vector.reciprocal(sr1[:, 0:8], sr1[:, 0:8])
        nc.vector.scalar_tensor_tensor(sr1[:, 8:16], mvv[:, :, 0], -1.0, sr1[:, 0:8],
                                       ALU.mult, ALU.mult)

        wB = es.enter_context(tc.tile_pool(name="wB", bufs=1))
        wstB = es.enter_context(tc.tile_pool(name="wstB", bufs=1))
        wf18 = wB.tile([128, FT, DE], fp8, name="wf18")
        wf28 = wB.tile([128, 16, 512], fp8, name="wf28")
        hT8 = wB.tile([128, 2, 2, N], fp8, name="hT8")
        wst = es.enter_context(tc.tile_pool(name="wstA", bufs=1))

        def stage_cast(dst3, src_dram, src_r0, src_c0, ncols, dst_c, dst_cols0,
                       scale, eng, nchunk=1):
            st = wst.tile([128, nchunk, ncols], f32, tag="wstage", name="wstage")
            dma(st[:], src_dram[src_r0:src_r0 + nchunk * 128,
                                src_c0:src_c0 + ncols]
                .rearrange("(c p) k -> p c k", p=128))
            for i in range(nchunk):
                o = dst3[:, dst_c + i, dst_cols0:dst_cols0 + ncols]
                if eng == "act":
                    nc.scalar.activation(o, st[:, i, :], AF.Copy, scale=float(scale))
                elif eng == "dve":
                    nc.vector.tensor_scalar(o, st[:, i, :], float(scale), None, ALU.mult)
                else:
                    nc.gpsimd.tensor_scalar(o, st[:, i, :], float(scale), None, ALU.mult)

        with tc.tile_pool(name="vwp", bufs=1) as vwp, \
             tc.tile_pool(name="wA", bufs=1) as wA:
            vw = [vwp.tile([128, NT, 512], fp8, name=f"vw_{h}", tag=f"vw_{h}") for h in range(H)]
            qkpair = [vwp.tile([128, N], fp8, name=f"pair_{p}", tag=f"pair_{p}")
                      for p in range(16)]
            wv8 = wA.tile([128, FT, 4096], fp8, name="wv8")
            wm8 = wA.tile([128, 32, 512], fp8, name="wm8")
            lnx8 = wA.tile([128, 2, 2, N], fp8, name="lnx8")

            with tc.tile_pool(name="lnst", bufs=3) as lnst:
                for nt in range(NT):
                    xn = lnst.tile([128, D], bf16, tag="xn", name="xn")
                    nc.scalar.activation(xn[:], xts[nt][:], AF.Identity,
                                         bias=sr1[:, 8 + nt:9 + nt], scale=sr1[:, nt:nt + 1])
                    xT = lnst.tile([128, FT, 128], bf16, tag="xT", name="xT")
                    dma(xT[:], xn[:], transpose=True)
                    for ft in range(FT):
                        o = lnx8[:, ft // 2, ft % 2, nt * 128:(nt + 1) * 128]
                        if ln_trivial:
                            nc.gpsimd.tensor_copy(o, xT[:, ft, :])
                        else:
                            nc.gpsimd.tensor_scalar(
                                o, xT[:, ft, :], ln1g_c[:, ft:ft + 1],
                                ln1b_c[:, ft:ft + 1], ALU.mult, ALU.add)

            # ---------- weight staging: qk first, then v, m (DMA order) ------
            with tc.tile_pool(name="wqkp", bufs=1) as wqkp:
                wqk8 = wqkp.tile([128, FT, 2048], fp8, name="wqk8", tag="wqk8")
                for ft in range(FT):
                    for cb in range(2):
                        eng = ("act", "act", "dve", "gps")[(ft * 2 + cb) % 4]
                        stage_cast(wqk8, Wqkv_d, ft * 128, cb * 1024, 1024, ft,
                                   cb * 1024, 32.0, eng)

                # ---------- qk projection -> pair tiles (q,k per head) -------
                for h in range(H):
                    for half in range(2):
                        p = half * 8 + h
                        pair = qkpair[p]
                        wl = wqk8[:, :, half * 1024:(half + 1) * 1024] \
                            .rearrange("p f (g c) -> p f g c", g=2)[:, :, :, h * 64:(h + 1) * 64]
                        for ch in range(2):
                            ps = ps_m.tile([128, 512], f32, tag="mm")
                            for j in range(2):
                                nc.tensor.matmul(ps[:], wl[:, 2 * j:2 * j + 2, :, :],
                                                 lnx8[:, j, :, ch * 512:(ch + 1) * 512],
                                                 start=(j == 0), stop=(j == 1), perf_mode=DR)
                            if p % 2 == 0:
                                nc.scalar.activation(pair[:, ch * 512:(ch + 1) * 512], ps[:],
                                                     AF.Identity, bias=bqp[:, p:p + 1])
                            else:
                                nc.vector.tensor_scalar(pair[:, ch * 512:(ch + 1) * 512],
                                                        ps[:], bqp[:, p:p + 1], None, ALU.add)


            # ================= attention + interleaved v_fm/VW =================
            exsc = float(SCALE / 1024.0)

            def emit_vfm_vw(h, vfmp):
                vfm = vfmp.tile([128, FT, N], fp8, tag="vfm", name=f"vfm_{h}")
                for sl in range(4):
                    wl = wv8[:, :, h * 512 + sl * 128:h * 512 + (sl + 1) * 128]
                    for ch in range(2):
                        ps = ps_m.tile([128, 512], f32, tag="mm")
                        for j in range(2):
                            nc.tensor.matmul(ps[:], wl[:, 2 * j:2 * j + 2, :],
                                             lnx8[:, j, :, ch * 512:(ch + 1) * 512],
                                             start=(j == 0), stop=(j == 1), perf_mode=DR)
                        nc.vector.tensor_scalar(vfm[:, sl, ch * 512:(ch + 1) * 512],
                                                ps[:], 1.0 / 32.0, None, ALU.mult)
                for mt in range(NT):
                    ps = ps_m.tile([128, 512], f32, tag="mm")
                    for c in range(2):
                        nc.tensor.matmul(
                            ps[:], vfm[:, 2 * c:2 * c + 2, mt * 128:(mt + 1) * 128],
                            wm8[:, h * 4 + 2 * c:h * 4 + 2 * c + 2, :],
                            start=(c == 0), stop=(c == 1), perf_mode=DR)
                    nc.vector.tensor_scalar(vw[h][:, mt, :], ps[:], 0.25, None, ALU.mult)

            with tc.tile_pool(name="ep", bufs=11) as ep, \
                 tc.tile_pool(name="dp", bufs=3) as dp, \
                 tc.tile_pool(name="dgp", bufs=6) as dgp, \
                 tc.tile_pool(name="wtp", bufs=30) as wtp, \
                 tc.tile_pool(name="vfm", bufs=1) as vfmp, \
                 tc.tile_pool(name="ps_s", bufs=2, space="PSUM") as ps_s, \
                 tc.tile_pool(name="aps", bufs=2, space="PSUM") as aps:

                def emit_lnf_batch(b, lnp):
                    mvv2 = mvf[:].rearrange("p (n c) -> p n c", c=2)
                    nc.scalar.activation(srf[:, 4 * b:4 * b + 4],
                                         mvv2[:, 4 * b:4 * b + 4, 1], AF.Sqrt,
                                         bias=eps_c[:])
                    nc.vector.reciprocal(srf[:, 4 * b:4 * b + 4], srf[:, 4 * b:4 * b + 4])
                    nc.vector.scalar_tensor_tensor(srf[:, 8 + 4 * b:12 + 4 * b],
                                                   mvv2[:, 4 * b:4 * b + 4, 0], -1.0,
                                                   srf[:, 4 * b:4 * b + 4],
                                                   ALU.mult, ALU.mult)
                    for nt in range(4 * b, 4 * b + 4):
                        hn = lnp.tile([128, D], bf16, tag="hn", name="hn", bufs=3)
                        nc.scalar.activation(hn[:], x2ts[nt][:], AF.Identity,
                                             bias=srf[:, 8 + nt:9 + nt],
                                             scale=srf[:, nt:nt + 1])
                        hT = lnp.tile([128, FT, 128], bf16, tag="hT", name="hT", bufs=3)
                        dma(hT[:], hn[:], transpose=True)
                        for ft in range(FT):
                            o = hT8[:, ft // 2, ft % 2, nt * 128:(nt + 1) * 128]
                            if ln_trivial:
                                nc.gpsimd.tensor_copy(o, hT[:, ft, :])
                            else:
                                nc.gpsimd.tensor_scalar(
                                    o, hT[:, ft, :], lnfg_c[:, ft:ft + 1],
                                    lnfb_c[:, ft:ft + 1], ALU.mult, ALU.add)

                wts_all = []

                def emit_attn_half(nt, half):
                    X = ps_m.tile([128, 512], f32, tag="mm")
                    wts_n = wts_all[nt]
                    hs = range(4 * half, 4 * half + 4)
                    for h in hs:
                        for tp_ in range(4):
                            nc.tensor.matmul(X[:], wts_n[h][:, 2 * tp_:2 * tp_ + 2, :],
                                             vw[h][:, 2 * tp_:2 * tp_ + 2, :],
                                             start=(h == 4 * half and tp_ == 0),
                                             stop=(h == 4 * half + 3 and tp_ == 3),
                                             perf_mode=DR)
                    if half == 0:
                        x2t = x2p.tile([128, D], bf16, name=f"x2_{nt}", tag="x2")
                        nc.vector.scalar_tensor_tensor(x2t[:], X[:], 2.0 ** -9,
                                                       xts[nt][:], ALU.mult, ALU.add)
                        x2ts.append(x2t)
                    else:
                        x2t = x2ts[nt]
                        nc.vector.scalar_tensor_tensor(x2t[:], X[:], 2.0 ** -9,
                                                       x2t[:], ALU.mult, ALU.add)
                        st6 = small.tile([128, 6], f32, tag="st6")
                        nc.vector.bn_stats(out=st6[:], in_=x2t[:])
                        nc.vector.bn_aggr(out=mvf[:, 2 * nt:2 * nt + 2], in_=st6[:])

                for nt in range(NT):
                    dcol = dp.tile([128, 16], f32, tag="dcol")
                    Es = []
                    for h in range(H):
                        E = ep.tile([128, 2, N], fp8, tag="E", name=f"E_{h}")
                        for br in range(2):
                            S = ps_s.tile([128, N], f32, tag="S")
                            for mc in range(2):
                                nc.tensor.matmul(
                                    S[:, mc * 512:(mc + 1) * 512],
                                    qkpair[h][br * 64:(br + 1) * 64,
                                              nt * 128:(nt + 1) * 128],
                                    qkpair[8 + h][br * 64:(br + 1) * 64,
                                                  mc * 512:(mc + 1) * 512],
                                    start=True, stop=True)
                            nc.scalar.activation(E[:, br, :], S[:], AF.Exp, scale=exsc,
                                                 accum_out=dcol[:, 8 * br + h:8 * br + h + 1])
                        Es.append(E)
                    if nt == 0:
                        # v/m weight casts: emitted here so they queue BEHIND
                        # nt=0 attention work on DVE/Pool (no head-of-line
                        # blocking of the exp stream on ACT)
                        for ft in range(FT):
                            for cb in range(4):
                                stage_cast(wv8, Wqkv_d, ft * 128, 2048 + cb * 1024,
                                           1024, ft, cb * 1024, 32.0, "dve")
                        for blk in range(16):
                            stage_cast(wm8, Wm_d, blk * 256, 0, 512, blk * 2, 0, 32.0,
                                       "gps", nchunk=2)
                    if nt == 1:
                        # bvW row via fp8 DR; fold into residual row; broadcast
                        bvw_ps = ps_m.tile([128, 512], f32, tag="mm")
                        for j in range(32):
                            nc.tensor.matmul(bvw_ps[0:1, :], bv8[:, j, :],
                                             wm8[:, j, :],
                                             start=(j == 0), stop=(j == 31))
                        nc.vector.scalar_tensor_tensor(row1[:], bvw_ps[0:1, :],
                                                       (1.0 - lam) / 1024.0,
                                                       row1[:], ALU.mult, ALU.add)
                        tp_b = ps_m.tile([128, 512], f32, tag="mm")
                        nc.tensor.matmul(tp_b[:], ones1[:], row1[:], start=True, stop=True)
                        nc.vector.tensor_copy(TP1[:], tp_b[:])
                        nc.vector.tensor_scalar(row21b[:], row21[:], 16.0, None, ALU.mult)
                        nc.vector.tensor_copy(ones1b[:], ones1[:])
                        for i in range(NT):
                            nc.gpsimd.tensor_add(xts[i][:], xts[i][:], TP1[:])
                    if nt == 0:
                        for hh in range(H):
                            emit_vfm_vw(hh, vfmp)
                    if nt == 5:
                        # ffn weights stream in during attention
                        for ft in range(FT):
                            for cb in range(2):
                                st = wstB.tile([128, 1024], f32, tag="wstB", name="wstB")
                                dma(st[:], Wf1_d[ft * 128:(ft + 1) * 128,
                                                 cb * 1024:(cb + 1) * 1024])
                                nc.gpsimd.tensor_scalar(
                                    wf18[:, ft, cb * 1024:(cb + 1) * 1024],
                                    st[:], 16.0, None, ALU.mult)
                        for blk in range(8):
                            st = wstB.tile([128, 2, 512], f32, tag="wstB", name="wstB2")
                            dma(st[:], Wf2_d[blk * 256:(blk + 1) * 256, :]
                                .rearrange("(c p) k -> p c k", p=128))
                            for i in range(2):
                                nc.gpsimd.tensor_scalar(wf28[:, blk * 2 + i, :],
                                                        st[:, i, :], 16.0, None, ALU.mult)
                    wts = []
                    wts_all.append(wts)
                    dview = dcol[:].rearrange("p (b h) -> p h b", b=2)
                    for h in range(H):
                        rc2 = dp.tile([128, 2], f32, tag="rc2", bufs=10)
                        nc.vector.reciprocal(rc2[:], dview[:, h, :])
                        dg = dgp.tile([128, 2, 128], fp8, tag="dg")
                        nc.vector.tensor_scalar(dg[:, 0, :], ident64[:], rc2[:, 0:1],
                                                None, ALU.mult)
                        nc.vector.tensor_scalar(dg[:, 1, :], identneg[:], rc2[:, 1:2],
                                                None, ALU.mult)
                        wt = wtp.tile([128, NT, 128], fp8, tag="wt", name=f"wt_{h}")
                        for hf in range(2):
                            ps = aps.tile([128, 512], f32, tag="a")
                            for c in range(4):
                                mt = hf * 4 + c
                                nc.tensor.matmul(
                                    ps[:, c * 128:(c + 1) * 128],
                                    Es[h][:, :, mt * 128:(mt + 1) * 128], dg[:],
                                    start=True, stop=True, perf_mode=DR)
                            nc.vector.tensor_copy(
                                wt[:, hf * 4:(hf + 1) * 4, :].rearrange("p c k -> p (c k)"),
                                ps[:])
                        wts.append(wt)
                    if nt >= 2:
                        emit_attn_half(nt - 2, 0)
                    if nt >= 3:
                        emit_attn_half(nt - 3, 1)
                    if nt == 7:
                        emit_lnf_batch(0, small)

                emit_attn_half(6, 0)
                emit_attn_half(5, 1)
                emit_attn_half(7, 0)
                emit_attn_half(6, 1)
                emit_attn_half(7, 1)

            # ---------- LNf batch 1 + FFN ----------
            with tc.tile_pool(name="wB2", bufs=1) as wB2, \
                 tc.tile_pool(name="lnst2", bufs=3) as lnst2, \
                 tc.tile_pool(name="yp", bufs=3) as yp:
                aT8 = wB2.tile([128, 16, N], fp8, name="aT8")
                mvv2 = mvf[:].rearrange("p (n c) -> p n c", c=2)
                nc.scalar.activation(srf[:, 4:8], mvv2[:, 4:8, 1], AF.Sqrt, bias=eps_c[:])
                nc.vector.reciprocal(srf[:, 4:8], srf[:, 4:8])
                nc.vector.scalar_tensor_tensor(srf[:, 12:16], mvv2[:, 4:8, 0], -1.0,
                                               srf[:, 4:8], ALU.mult, ALU.mult)
                for nt in range(4, NT):
                    hn = lnst2.tile([128, D], bf16, tag="hn", name="hn")
                    nc.scalar.activation(hn[:], x2ts[nt][:], AF.Identity,
                                         bias=srf[:, 8 + nt:9 + nt],
                                         scale=srf[:, nt:nt + 1])
                    hT = lnst2.tile([128, FT, 128], bf16, tag="hT", name="hT")
                    dma(hT[:], hn[:], transpose=True)
                    for ft in range(FT):
                        o = hT8[:, ft // 2, ft % 2, nt * 128:(nt + 1) * 128]
                        if ln_trivial:
                            nc.gpsimd.tensor_copy(o, hT[:, ft, :])
                        else:
                            nc.gpsimd.tensor_scalar(
                                o, hT[:, ft, :], lnfg_c[:, ft:ft + 1],
                                lnfb_c[:, ft:ft + 1], ALU.mult, ALU.add)

                for s in range(16):
                    for ch in range(2):
                        ps = ps_m.tile([128, 512], f32, tag="mm")
                        for j in range(2):
                            nc.tensor.matmul(ps[:],
                                             wf18[:, 2 * j:2 * j + 2, s * 128:(s + 1) * 128],
                                             hT8[:, j, :, ch * 512:(ch + 1) * 512],
                                             start=(j == 0), stop=(j == 1), perf_mode=DR)
                        nc.scalar.activation(aT8[:, s, ch * 512:(ch + 1) * 512], ps[:],
                                             AF.Silu, bias=bf1_c[:, s:s + 1], scale=1.0 / 16.0)

                for nt in range(NT):
                    ps = ps_m.tile([128, 512], f32, tag="mm")
                    nc.tensor.matmul(ps[:], ones1b[:], row21b[:], start=True, stop=False)
                    for u in range(8):
                        nc.tensor.matmul(ps[:],
                                         aT8[:, 2 * u:2 * u + 2, nt * 128:(nt + 1) * 128],
                                         wf28[:, 2 * u:2 * u + 2, :],
                                         start=False, stop=(u == 7), perf_mode=DR)
                    yt = yp.tile([128, D], f32, tag="y")
                    nc.vector.scalar_tensor_tensor(yt[:], ps[:], 1.0 / 16.0,
                                                   x2ts[nt][:], ALU.mult, ALU.add)
                    dma(y_d[nt * 128:(nt + 1) * 128, :], yt[:])


_NC_CACHE = {}


def _get_nc(lam: float, ln_trivial: bool = False):
    key = (float(lam), bool(ln_trivial))
    if key not in _NC_CACHE:
        _NC_CACHE[key] = build_program(key[0], key[1])
    return _NC_CACHE[key]


def kernel(**inputs) -> np.ndarray:
    from concourse.bass_utils import run_bass_kernel_spmd

    lam = float(np.asarray(inputs["lam"]))
    ln_trivial = all((
        np.allclose(np.asarray(inputs["ln1_g"]), 1.0),
        np.allclose(np.asarray(inputs["ln1_b"]), 0.0),
        np.allclose(np.asarray(inputs["lnf_g"]), 1.0),
        np.allclose(np.asarray(inputs["lnf_b"]), 0.0),
    ))
    nc = _get_nc(lam, ln_trivial)
    names = ["Wqkv", "bqkv", "Wm", "bm", "Wt1", "bt1", "Wt2", "bt2",
             "Wf1", "bf1", "Wf2", "bf2", "ln1_g", "ln1_b", "lnf_g", "lnf_b"]
    shared = {n: np.ascontiguousarray(np.asarray(inputs[n], dtype=np.float32)) for n in names}
    x = np.asarray(inputs["x"], dtype=np.float32)
    t = np.asarray(inputs["t"], dtype=np.float32)
    in_maps = []
    for b in range(B):
        m = dict(shared)
        m["x"] = np.ascontiguousarray(x[b])
        m["t"] = np.ascontiguousarray(t[b])
        in_maps.append(m)
    res = run_bass_kernel_spmd(nc, in_maps, core_ids=list(range(B)))
    return np.stack([res.results[b]["y"] for b in range(B)], axis=0).astype(np.float32)
